# revision 1
# baseline (speedup 1.0000x reference)
"""Trainium2 Bass kernel for ConditionalCrossAttentionBlock (8 NeuronCores).

Sharding: 2 batch groups x 4-way query-sequence data-parallel. Core c
handles batch b=c//4, query tokens [(c%4)*1024, (c%4+1)*1024). The small
condition y (and the k/v projections) are replicated inside each batch
group; every core computes its output slice independently (no
collectives) and the host concatenates the 8 disjoint slices.

On-device layout is feature-major ("transposed"): the host pre-transposes
the x slice and all weight matrices so every matmul contracts over the
SBUF partition dim. All big matmuls run as float32r (full PE rate at
free-dim 512). rotate_half (RoPE) is a cross-partition shift, so it is
done on the PE with a signed permutation matrix. Softmax skips the
running-max (scores are bounded by ~sqrt(128) after RMSNorm) and gets
row sums via a ones-vector matmul riding the PV accumulation.
"""

import sys

sys.path.insert(0, "/opt/trn_rl_repo")

import numpy as np
from contextlib import ExitStack

import concourse.bass as bass
import concourse.bacc as bacc
import concourse.tile as tile
import concourse.mybir as mybir
from concourse.bass_utils import run_bass_kernel_spmd
from concourse.masks import make_identity

F32 = mybir.dt.float32
F32R = mybir.dt.float32r
AX = mybir.AxisListType
OP = mybir.AluOpType
AF = mybir.ActivationFunctionType

DIM = 2048
KV_DIM = 1024
H = 16
DH = 128
B = 2
LQ = 4096
LKV = 1024
QTOK = 1024          # query tokens per core
N_CORES = 8
EPS = 1e-6
SCALE = 1.0 / float(np.sqrt(DH))

P = 128
FT = DIM // P        # 16 of-tiles of the 2048 dim
KT = KV_DIM // P     # 8 tiles of the 1024 kv feature dim
RT = LKV // P        # 8 kv-token tiles
QC = QTOK // 512     # 2 query chunks of 512


def _r(ap):
    return ap.bitcast(F32R)


def _build_body(ctx, nc, tc, io):
    const = ctx.enter_context(tc.tile_pool(name="const", bufs=1))
    ident = const.tile([P, P], F32, name="ident")
    make_identity(nc, ident[:])
    ones_f = const.tile([P, 1], F32, name="ones_f")
    nc.gpsimd.memset(ones_f[:], 1.0)
    ones = const.tile([P, 1], F32R, name="ones")
    nc.scalar.copy(ones[:], ones_f[:])
    rotT = const.tile([P, P], F32R, name="rotT")
    nc.sync.dma_start(rotT[:], io["rotT"][:, :])
    eps_col = const.tile([P, 1], F32, name="eps_col")
    nc.gpsimd.memset(eps_col[:], EPS)

    bq_c = const.tile([P, FT], F32, name="bq_c")
    bk_c = const.tile([P, FT], F32, name="bk_c")
    bo_c = const.tile([P, FT], F32, name="bo_c")
    wq_c = const.tile([P, FT], F32, name="wq_c")
    wk_c = const.tile([P, FT], F32, name="wk_c")
    for dst, src in ((bq_c, io["bq"]), (bk_c, io["bk"]), (bo_c, io["bo"]),
                     (wq_c, io["rmsq_w"]), (wk_c, io["rmsk_w"])):
        nc.gpsimd.dma_start(dst[:], src[:, :].rearrange("(m p) o -> p (m o)", p=P))


    cosq = const.tile([P, QTOK], F32, name="cosq")
    sinq = const.tile([P, QTOK], F32, name="sinq")
    cosk = const.tile([P, LKV], F32, name="cosk")
    sink = const.tile([P, LKV], F32, name="sink")
    nc.sync.dma_start(cosq[:], io["cosqT"][:, :])
    nc.sync.dma_start(sinq[:], io["sinqT"][:, :])
    nc.sync.dma_start(cosk[:], io["coskT"][:, :])
    nc.sync.dma_start(sink[:], io["sinkT"][:, :])

    rstdq_row = const.tile([1, QTOK], F32, name="rstdq_row")
    rstdk_row = const.tile([1, LKV], F32, name="rstdk_row")

    # ---- generic of-tiled projection with DRAM spill + rms stats -------
    def proj_spill(w_dram, x_tiles, n_k, L, bias_c, w_c, spill, ssq_psum,
                   sq_pool, wslice_pool, out_pool, ps_pool, tag,
                   x_loads=None):
        nch = L // 512
        for mg in range(FT // 4):
            wts = []
            for i in range(n_k):
                if mg == 0 and x_loads is not None and i < len(x_loads):
                    xt, xsrc = x_loads[i]
                    nc.sync.dma_start(xt[:], xsrc)
                wt = wslice_pool.tile([P, 512], F32R, name=f"w{tag}",
                                      tag=f"w{tag}{i}")
                nc.sync.dma_start(
                    wt[:], w_dram[i * P:(i + 1) * P, mg * 512:(mg + 1) * 512])
                wts.append(wt)
            if mg == 0 and x_loads is not None:
                for xt, xsrc in x_loads[n_k:]:
                    nc.sync.dma_start(xt[:], xsrc)
            for ml in range(4):
                m = mg * 4 + ml
                ps = ps_pool.tile([P, L], F32, name=f"ps{tag}")
                for qc in range(nch):
                    for i in range(n_k):
                        nc.tensor.matmul(
                            ps[:, qc * 512:(qc + 1) * 512],
                            _r(wts[i][:, ml * P:(ml + 1) * P]),
                            _r(x_tiles[i][:, qc * 512:(qc + 1) * 512]),
                            start=(i == 0), stop=(i == n_k - 1))
                # stats on raw (post-bias, pre-rms-weight) values
                sq = sq_pool.tile([P, L], F32R, name=f"sq{tag}")
                nc.scalar.activation(sq[:], ps[:], AF.Square,
                                     bias=bias_c[:, m:m + 1])
                for qc in range(nch):
                    nc.tensor.matmul(
                        ssq_psum[:, qc * 512:(qc + 1) * 512],
                        _r(ones[:]), _r(sq[:, qc * 512:(qc + 1) * 512]),
                        start=(m == 0), stop=(m == FT - 1),
                        skip_group_check=True)
                ow = out_pool.tile([P, L], F32R, name=f"ow{tag}")
                nc.vector.tensor_scalar(
                    ow[:], ps[:], bias_c[:, m:m + 1], w_c[:, m:m + 1],
                    op0=OP.add, op1=OP.mult)
                nc.gpsimd.dma_start(spill[m * P:(m + 1) * P, :], ow[:])

    # ---------------- Phases A (LayerNorm y), Q, K, V -------------------
    with tc.tile_pool(name="xT", bufs=1) as xT_pool, \
         tc.tile_pool(name="ynT", bufs=1) as ynT_pool:
        ynT = [ynT_pool.tile([P, LKV], F32R, name=f"ynT{i}") for i in range(KT)]

        with tc.tile_pool(name="phA", bufs=2) as pA, \
             tc.tile_pool(name="phA_yn", bufs=1) as pYn, \
             tc.tile_pool(name="phA_ps", bufs=2, space="PSUM") as pAp, \
             tc.tile_pool(name="phA_sm", bufs=2) as pAs:
            lnw_b = pYn.tile([P, KV_DIM], F32, name="lnw_b")
            lnb_b = pYn.tile([P, KV_DIM], F32, name="lnb_b")
            t = pAs.tile([1, KV_DIM], F32, name="lnw_row", tag="lnrow")
            nc.sync.dma_start(t[:], io["ln_w"][:, :])
            nc.gpsimd.partition_broadcast(lnw_b[:], t[:])
            t = pAs.tile([1, KV_DIM], F32, name="lnb_row", tag="lnrow")
            nc.sync.dma_start(t[:], io["ln_b"][:, :])
            nc.gpsimd.partition_broadcast(lnb_b[:], t[:])
            yn_rows = []
            ssqs, rstds = [], []
            for rr in range(RT):
                y_r = pA.tile([P, KV_DIM], F32, name="y_r")
                nc.sync.dma_start(y_r[:], io["y"][rr * P:(rr + 1) * P, :])
                ssum = pAs.tile([P, 1], F32, name="ssum")
                nc.vector.tensor_reduce(ssum[:], y_r[:], axis=AX.X, op=OP.add)
                mean = pAs.tile([P, 1], F32, name="mean")
                nc.vector.tensor_scalar(mean[:], ssum[:], 1.0 / KV_DIM, None,
                                        op0=OP.mult)
                yc = pYn.tile([P, KV_DIM], F32, name=f"yc{rr}")
                nc.vector.tensor_scalar(yc[:], y_r[:], mean[:], None,
                                        op0=OP.subtract)
                sq = pA.tile([P, KV_DIM], F32, name="sq")
                ssq = pAs.tile([P, 1], F32, name=f"ssq{rr}", tag=f"ssq{rr}")
                nc.scalar.activation(sq[:], yc[:], AF.Square, accum_out=ssq[:])
                ssqs.append(ssq)
                yn_rows.append(yc)
            lnvs = []
            for rr in range(RT):
                lnv = pAs.tile([P, 1], F32, name=f"lnv{rr}", tag=f"lnv{rr}")
                nc.scalar.activation(lnv[:], ssqs[rr][:], AF.Ln,
                                     scale=1.0 / KV_DIM, bias=eps_col[:])
                lnvs.append(lnv)
            for rr in range(RT):
                rstd = pAs.tile([P, 1], F32, name=f"rstd{rr}", tag=f"rstd{rr}")
                nc.scalar.activation(rstd[:], lnvs[rr][:], AF.Exp, scale=-0.5)
                rstds.append(rstd)
            for rr in range(RT):
                t1 = pA.tile([P, KV_DIM], F32, name="t1")
                nc.vector.scalar_tensor_tensor(
                    t1[:], yn_rows[rr][:], rstds[rr][:], lnw_b[:],
                    op0=OP.mult, op1=OP.mult)
                nc.vector.tensor_tensor(yn_rows[rr][:], t1[:], lnb_b[:],
                                        op=OP.add)

            for fi in range(KT):
                for rg in range(RT // 4):
                    ps = pAp.tile([P, 512], F32, name="tr_ps")
                    for j in range(4):
                        rr = rg * 4 + j
                        nc.tensor.transpose(
                            ps[:, j * P:(j + 1) * P],
                            yn_rows[rr][:, fi * P:(fi + 1) * P], ident[:])
                    nc.scalar.copy(ynT[fi][:, rg * 512:(rg + 1) * 512], ps[:])

        # ---------------- Phase Q: q projection, spilled --------------------
        with tc.tile_pool(name="phQ_w", bufs=1) as wpool, \
             tc.tile_pool(name="phQ_sq", bufs=2) as sqpool, \
             tc.tile_pool(name="phQ_out", bufs=2) as opool, \
             tc.tile_pool(name="phQ_ps", bufs=3, space="PSUM") as pspool, \
             tc.tile_pool(name="phQ_ssq", bufs=1, space="PSUM") as ssqpool:
            xT = [xT_pool.tile([P, QTOK], F32R, name=f"xT{i}") for i in range(FT)]
            x_loads = [(xT[i], io["xT"][i * P:(i + 1) * P, :])
                       for i in range(FT)]
            ssq_q = ssqpool.tile([1, QTOK], F32, name="ssq_q")
            proj_spill(io["WqT"], xT, FT, QTOK, bq_c, wq_c, io["q_spill"],
                       ssq_q, sqpool, wpool, opool, pspool, "q",
                       x_loads=x_loads)
            lnq = sqpool.tile([1, QTOK], F32, name="lnq")
            nc.scalar.activation(lnq[:], ssq_q[:], AF.Ln, scale=1.0 / DIM,
                                 bias=eps_col[0:1, :])
            nc.scalar.activation(rstdq_row[:], lnq[:], AF.Exp, scale=-0.5)

        # fold rstd into the rope cos/sin tables (rstd commutes with rope)
        with tc.tile_pool(name="fold", bufs=1) as fold:
            rbq = fold.tile([P, QTOK], F32, name="rbq")
            nc.gpsimd.partition_broadcast(rbq[:], rstdq_row[:])
            nc.vector.tensor_tensor(cosq[:], cosq[:], rbq[:], op=OP.mult)
            nc.vector.tensor_tensor(sinq[:], sinq[:], rbq[:], op=OP.mult)

        # Phase K: k projection, spilled
        with tc.tile_pool(name="phK_w", bufs=1) as wpool, \
             tc.tile_pool(name="phK_sq", bufs=2) as sqpool, \
             tc.tile_pool(name="phK_out", bufs=2) as opool, \
             tc.tile_pool(name="phK_ps", bufs=3, space="PSUM") as pspool, \
             tc.tile_pool(name="phK_ssq", bufs=1, space="PSUM") as ssqpool:
            ssq_k = ssqpool.tile([1, LKV], F32, name="ssq_k")
            proj_spill(io["WkT"], ynT, KT, LKV, bk_c, wk_c, io["k_spill"],
                       ssq_k, sqpool, wpool, opool, pspool, "k")
            lnk = sqpool.tile([1, LKV], F32, name="lnk")
            nc.scalar.activation(lnk[:], ssq_k[:], AF.Ln,
                                 scale=1.0 / DIM, bias=eps_col[0:1, :])
            nc.scalar.activation(rstdk_row[:], lnk[:], AF.Exp, scale=-0.5)

        with tc.tile_pool(name="foldk", bufs=1) as foldk:
            rbk = foldk.tile([P, LKV], F32, name="rbk")
            nc.gpsimd.partition_broadcast(rbk[:], rstdk_row[:])
            nc.vector.tensor_tensor(cosk[:], cosk[:], rbk[:], op=OP.mult)
            nc.vector.tensor_tensor(sink[:], sink[:], rbk[:], op=OP.mult)

        # Phase V: v projection (natural layout), spilled
        with tc.tile_pool(name="phV_ps", bufs=4, space="PSUM") as pspool, \
             tc.tile_pool(name="phV_out", bufs=3) as vout, \
             tc.tile_pool(name="phV_bv", bufs=1) as pbv:
            bv_b = pbv.tile([P, DIM], F32, name="bv_b")
            t = vout.tile([1, DIM], F32, name="bv_row", tag="bvrow")
            nc.sync.dma_start(t[:], io["bv"][:, :])
            nc.gpsimd.partition_broadcast(bv_b[:], t[:])
            for half in range(2):
                with tc.tile_pool(name=f"phV_w{half}", bufs=1) as wpool:
                    wvh = []
                    for i in range(KT):
                        wt = wpool.tile([P, 1024], F32R, name=f"wv{i}")
                        nc.sync.dma_start(
                            wt[:], io["WvT"][i * P:(i + 1) * P,
                                             half * 1024:(half + 1) * 1024])
                        wvh.append(wt)
                    for rr in range(RT):
                        for oc in range(2):
                            ps = pspool.tile([P, 512], F32, name="ps_v")
                            for i in range(KT):
                                nc.tensor.matmul(
                                    ps[:],
                                    _r(ynT[i][:, rr * P:(rr + 1) * P]),
                                    _r(wvh[i][:, oc * 512:(oc + 1) * 512]),
                                    start=(i == 0), stop=(i == KT - 1))
                            off = half * 1024 + oc * 512
                            vs = vout.tile([P, 512], F32R, name="vs")
                            nc.vector.tensor_tensor(
                                vs[:], ps[:], bv_b[:, off:off + 512],
                                op=OP.add)
                            nc.gpsimd.dma_start(
                                io["v_spill"][rr * P:(rr + 1) * P,
                                              off:off + 512], vs[:])


    # ---------------- Phase ATT -----------------------------------------
    att_pool = ctx.enter_context(tc.tile_pool(name="attn_out", bufs=1))
    attn_outT = [att_pool.tile([P, QTOK], F32R, name=f"ao{h}")
                 for h in range(H)]

    with tc.tile_pool(name="T_kq", bufs=2) as kqpool, \
         tc.tile_pool(name="T_rp", bufs=2) as rppool, \
         tc.tile_pool(name="T_kqr", bufs=2) as kqrpool, \
         tc.tile_pool(name="T_v", bufs=2) as vpool, \
         tc.tile_pool(name="T_pr", bufs=9) as probpool, \
         tc.tile_pool(name="T_sm", bufs=2) as smpool, \
         tc.tile_pool(name="T_sc_ps", bufs=2, space="PSUM") as scps, \
         tc.tile_pool(name="T_o_ps", bufs=1, space="PSUM") as ops_, \
         tc.tile_pool(name="T_s_ps", bufs=1, space="PSUM") as sps_:

        def rope(dst, src, cos_t, sin_t, L):
            # dst = src*cos + (R~ @ src)*sin ; rstd already folded in cos/sin
            nc.vector.tensor_tensor(dst[:], src[:].bitcast(F32), cos_t[:],
                                    op=OP.mult)
            for ch in range(L // 512):
                sl = slice(ch * 512, (ch + 1) * 512)
                rot = scps.tile([P, QTOK], F32, name="sc_ps", tag="sc_ps")
                nc.tensor.matmul(rot[:, 0:512], _r(rotT[:]), _r(src[:, sl]),
                                 start=True, stop=True)
                t2 = rppool.tile([P, 512], F32, name="rope_t2", tag="rope_t2")
                nc.vector.tensor_tensor(t2[:], rot[:, 0:512], sin_t[:, sl],
                                        op=OP.mult)
                nc.vector.tensor_tensor(dst[:, sl], dst[:, sl].bitcast(F32),
                                        t2[:], op=OP.add)

        def load_and_rope(h):
            kw = kqpool.tile([P, LKV], F32R, name="kw", tag="kw")
            nc.sync.dma_start(kw[:], io["k_spill"][h * P:(h + 1) * P, :])
            k_h = kqrpool.tile([P, LKV], F32R, name="k_h", tag="k_h")
            rope(k_h, kw, cosk, sink, LKV)
            qw = kqpool.tile([P, QTOK], F32R, name="qw", tag="qw")
            nc.sync.dma_start(qw[:], io["q_spill"][h * P:(h + 1) * P, :])
            q_h = kqrpool.tile([P, QTOK], F32R, name="q_h", tag="q_h")
            rope(q_h, qw, cosq, sinq, QTOK)
            vh = []
            for rr in range(RT):
                vt = vpool.tile([P, P], F32R, name="v_h", tag=f"v_h{rr}")
                nc.sync.dma_start(
                    vt[:], io["v_spill"][rr * P:(rr + 1) * P,
                                         h * P:(h + 1) * P])
                vh.append(vt)
            return k_h, q_h, vh

        cur = load_and_rope(0)
        for h in range(H):
            k_h, q_h, vh = cur
            o_ps = ops_.tile([P, QTOK], F32, name="o_ps")
            s_ps = sps_.tile([1, QTOK], F32, name="s_ps")
            probs = []
            for rr in range(RT):
                sc = scps.tile([P, QTOK], F32, name="sc_ps", tag="sc_ps")
                for qc in range(QC):
                    qsl = slice(qc * 512, (qc + 1) * 512)
                    nc.tensor.matmul(
                        sc[:, qsl], _r(k_h[:, rr * P:(rr + 1) * P]),
                        _r(q_h[:, qsl]), start=True, stop=True)
                pr = probpool.tile([P, QTOK], F32R, name="probs")
                nc.scalar.activation(pr[:], sc[:], AF.Exp, scale=SCALE)
                probs.append(pr)
            # next head's load+rope emitted here: its rot matmuls slot in
            # before this head's PV stream and the rope DVE work overlaps it
            if h + 1 < H:
                cur = load_and_rope(h + 1)
            for rr in range(RT):
                for qc in range(QC):
                    qsl = slice(qc * 512, (qc + 1) * 512)
                    nc.tensor.matmul(
                        o_ps[:, qsl], _r(vh[rr][:]), _r(probs[rr][:, qsl]),
                        start=(rr == 0), stop=(rr == RT - 1),
                        skip_group_check=True)
                    nc.tensor.matmul(
                        s_ps[:, qsl], _r(ones[:]), _r(probs[rr][:, qsl]),
                        start=(rr == 0), stop=(rr == RT - 1),
                        skip_group_check=True)
            nc.scalar.copy(attn_outT[h][:], o_ps[:])
            rs = smpool.tile([1, QTOK], F32, name="rs", tag="rs")
            nc.vector.reciprocal(rs[:], s_ps[:])
            rb = smpool.tile([P, QTOK], F32, name="rb", tag="rb")
            nc.gpsimd.partition_broadcast(rb[:], rs[:])
            nc.gpsimd.tensor_tensor(attn_outT[h][:],
                                    attn_outT[h][:].bitcast(F32), rb[:],
                                    op=OP.mult)

    # ---------------- Phase O: output projection ------------------------
    with tc.tile_pool(name="phO_w", bufs=1) as wpool, \
         tc.tile_pool(name="phO_out", bufs=3) as opool, \
         tc.tile_pool(name="phO_ps", bufs=4, space="PSUM") as pspool:
        for mg in range(FT // 4):
            wts = []
            for i in range(FT):
                wt = wpool.tile([P, 512], F32R, name="w_o", tag=f"w_o{i}")
                nc.sync.dma_start(
                    wt[:], io["WoT"][i * P:(i + 1) * P,
                                     mg * 512:(mg + 1) * 512])
                wts.append(wt)
            for ml in range(4):
                m = mg * 4 + ml
                for qc in range(QC):
                    ps = pspool.tile([P, 512], F32, name="ps_o")
                    for i in range(FT):
                        nc.tensor.matmul(
                            ps[:],
                            _r(wts[i][:, ml * P:(ml + 1) * P]),
                            _r(attn_outT[i][:, qc * 512:(qc + 1) * 512]),
                            start=(i == 0), stop=(i == FT - 1))
                    ow = opool.tile([P, 512], F32, name="ow_o")
                    nc.vector.tensor_scalar(
                        ow[:], ps[:], bo_c[:, m:m + 1], None, op0=OP.add)
                    nc.sync.dma_start(
                        io["out"][m * P:(m + 1) * P,
                                  qc * 512:(qc + 1) * 512], ow[:])


_CACHED_NC = None


def _build_nc():
    global _CACHED_NC
    if _CACHED_NC is not None:
        return _CACHED_NC
    nc = bacc.Bacc("TRN2")
    io = {}

    def inp(name, shape, dt=F32):
        io[name] = nc.declare_dram_parameter(name, list(shape), dt,
                                             isOutput=False)

    inp("xT", (DIM, QTOK), F32R)
    inp("y", (LKV, KV_DIM))
    inp("cosqT", (DH, QTOK))
    inp("sinqT", (DH, QTOK))
    inp("coskT", (DH, LKV))
    inp("sinkT", (DH, LKV))
    inp("WqT", (DIM, DIM), F32R)
    inp("WkT", (KV_DIM, DIM), F32R)
    inp("WvT", (KV_DIM, DIM), F32R)
    inp("WoT", (DIM, DIM), F32R)
    inp("bq", (DIM, 1))
    inp("bk", (DIM, 1))
    inp("bv", (1, DIM))
    inp("bo", (DIM, 1))
    inp("rmsq_w", (DIM, 1))
    inp("rmsk_w", (DIM, 1))
    inp("ln_w", (1, KV_DIM))
    inp("ln_b", (1, KV_DIM))
    inp("rotT", (P, P), F32R)
    io["out"] = nc.declare_dram_parameter("out", [DIM, QTOK], F32,
                                          isOutput=True)
    io["q_spill"] = nc.dram_tensor("q_spill", [DIM, QTOK], F32R)
    io["k_spill"] = nc.dram_tensor("k_spill", [DIM, LKV], F32R)
    io["v_spill"] = nc.dram_tensor("v_spill", [LKV, DIM], F32R)

    with tile.TileContext(nc) as tc:
        with ExitStack() as ctx:
            _build_body(ctx, nc, tc, io)
    nc.compile()
    _CACHED_NC = nc
    return nc


def _rot_matrix():
    # lhsT for rotate_half: matmul computes lhsT.T @ x = R~ @ x with
    # R~[d, d+64] = -1 (d < 64), R~[d, d-64] = +1 (d >= 64)
    R = np.zeros((P, P), np.float32)
    R[np.arange(64), np.arange(64) + 64] = -1.0
    R[np.arange(64) + 64, np.arange(64)] = 1.0
    return np.ascontiguousarray(R.T)


def _make_in_maps(x, y, x_cos, x_sin, y_cos, y_sin, Wq, bq, Wk, bk, Wv, bv,
                  Wo, bo, rmsq_w, rmsk_w, ln_w, ln_b):
    f = np.float32
    shared = dict(
        WqT=np.ascontiguousarray(np.asarray(Wq, f).T),
        WkT=np.ascontiguousarray(np.asarray(Wk, f).T),
        WvT=np.ascontiguousarray(np.asarray(Wv, f).T),
        WoT=np.ascontiguousarray(np.asarray(Wo, f).T),
        bq=np.asarray(bq, f).reshape(DIM, 1),
        bk=np.asarray(bk, f).reshape(DIM, 1),
        bv=np.asarray(bv, f).reshape(1, DIM),
        bo=np.asarray(bo, f).reshape(DIM, 1),
        rmsq_w=np.asarray(rmsq_w, f).reshape(DIM, 1),
        rmsk_w=np.asarray(rmsk_w, f).reshape(DIM, 1),
        ln_w=np.asarray(ln_w, f).reshape(1, KV_DIM),
        ln_b=np.asarray(ln_b, f).reshape(1, KV_DIM),
        rotT=_rot_matrix(),
    )
    in_maps = []
    for c in range(N_CORES):
        b, s = c // 4, c % 4
        t0 = s * QTOK
        m = dict(shared)
        m["xT"] = np.ascontiguousarray(np.asarray(x[b, t0:t0 + QTOK], f).T)
        m["y"] = np.ascontiguousarray(np.asarray(y[b], f))
        m["cosqT"] = np.ascontiguousarray(np.asarray(x_cos[b, t0:t0 + QTOK], f).T)
        m["sinqT"] = np.ascontiguousarray(np.asarray(x_sin[b, t0:t0 + QTOK], f).T)
        m["coskT"] = np.ascontiguousarray(np.asarray(y_cos[b], f).T)
        m["sinkT"] = np.ascontiguousarray(np.asarray(y_sin[b], f).T)
        in_maps.append(m)
    return in_maps


def kernel(**inputs) -> np.ndarray:
    nc = _build_nc()
    in_maps = _make_in_maps(**inputs)
    res = run_bass_kernel_spmd(nc, in_maps, core_ids=list(range(N_CORES)))
    out = np.empty((B, LQ, DIM), np.float32)
    for c in range(N_CORES):
        b, s = c // 4, c % 4
        out[b, s * QTOK:(s + 1) * QTOK, :] = res.results[c]["out"].T
    return out

